# revision 1
# baseline (speedup 1.0000x reference)
"""nn_AttentionOut_63711544869147 — causal multi-head attention + output projection,
distributed over 8 Trainium2 NeuronCores.

Module: out = softmax(causal(Q K^T / sqrt(d))) V @ W_O + b_O, returned with the
(unchanged) residual: reference returns the tuple (residual, out).

Sharding (8 cores = 2 batches x 4 head-groups of 4 heads, SPMD single program):
  each core computes full causal attention for its batch over its 4 heads and
  a partial projection  sum_{h in group} z_h @ W_O[h]  ->  pout [2048, 1024].
  The host sums the 4 head-group partials per batch (the "all-reduce" of the
  row-sharded W_O product), adds b_O, and passes the residual through.

Device dataflow per (head, 512-wide q strip), exact causal tiling:
  scores_T[kv,q] = K_h^T_tile.T @ Q_h^T_strip          (PE, bf16 inputs)
  expP = exp(scores_T * 1/8)                            (ACT; scale folds 1/sqrt(64))
  four diagonal kv tiles: partial-range matmul/exp from the first valid
  column, triangular mask on the leading 128 block (DVE)
  z_ext[65,q] += V_ext_tile.T @ expP                    (PE accum; V_ext = [V | 1]
                                                         so row 64 = softmax denom)
  z = z_ext[0:64] * (1/z_ext[64])  (DVE approx-reciprocal + DRAM-bounce
                                    broadcast DMA; fp32 denominators throughout)
  pout strip = z_T @ W_O_group                          (PE, 256-deep contraction)
"""

import numpy as np

import concourse.bass as bass
import concourse.bacc as bacc
import concourse.tile as tile
from concourse import mybir
from concourse.bass_utils import run_bass_kernel_spmd

F32 = mybir.dt.float32
F32R = mybir.dt.float32r
BF16 = mybir.dt.bfloat16

N_CORES = 8
N_HEADS = 16
H = 4          # heads per core
S = 2048
D = 64
P = 128
D_MODEL = 1024
NSTRIP = 4     # q strips of 512
QW = 512       # strip width

USE_F32R = True
USE_BF16 = True

_PROGRAM = None
LAST_RESULTS = None


def build_program(use_f32r=USE_F32R, use_bf16=USE_BF16):
    MMDT = BF16 if use_bf16 else (F32R if use_f32r else F32)
    nc = bacc.Bacc(target_bir_lowering=False)

    qT = nc.dram_tensor("qT", [H, D, S], MMDT, kind="ExternalInput")
    kT = nc.dram_tensor("kT", [H, D, S], MMDT, kind="ExternalInput")
    v = nc.dram_tensor("v", [H, S, D], MMDT, kind="ExternalInput")
    wo = nc.dram_tensor("wo", [2 * P, D_MODEL], MMDT, kind="ExternalInput")
    tri = nc.dram_tensor("tri", [P, P], MMDT, kind="ExternalInput")
    pout = nc.dram_tensor("pout", [S, D_MODEL], F32, kind="ExternalOutput")

    with tile.TileContext(nc) as tc:
        with (
            tc.tile_pool(name="persist", bufs=1) as persist,
            tc.tile_pool(name="expp", bufs=4) as expp,
            tc.tile_pool(name="rcpp", bufs=2) as rcpp,
            tc.tile_pool(name="outp", bufs=6) as outp,
            tc.tile_pool(name="znp", bufs=2) as znp,
            tc.tile_pool(name="scps", bufs=2, space="PSUM") as scps,
            tc.tile_pool(name="zps", bufs=3, space="PSUM") as zps,
            tc.tile_pool(name="wops", bufs=1, space="PSUM") as wops,
            tc.tile_pool(name="dramp", bufs=2, space="DRAM") as dramp,
        ):
            # ---- persistent loads (everything stays SBUF-resident) ----
            qT_sb = []
            kT_sb = []
            wo_sb = []
            for j in range(2):  # head pairs on the partition axis
                qt = persist.tile([P, S], MMDT, tag=f"qT{j}", name=f"qT{j}")
                nc.sync.dma_start(qt[:], qT[2 * j : 2 * j + 2].rearrange("h d s -> (h d) s"))
                qT_sb.append(qt)
                kt = persist.tile([P, S], MMDT, tag=f"kT{j}", name=f"kT{j}")
                nc.sync.dma_start(kt[:], kT[2 * j : 2 * j + 2].rearrange("h d s -> (h d) s"))
                kT_sb.append(kt)
                wt = persist.tile([P, D_MODEL], MMDT, tag=f"wo{j}", name=f"wo{j}")
                nc.sync.dma_start(wt[:], wo[P * j : P * (j + 1), :])
                wo_sb.append(wt)

            vext_sb = []
            for h in range(H):
                vt = persist.tile([P, 16, D + 1], MMDT, tag=f"vext{h}", name=f"vext{h}")
                nc.vector.memset(vt[:, :, D : D + 1] if use_bf16 else vt.bitcast(F32)[:, :, D : D + 1], 1.0)
                nc.sync.dma_start(vt[:, :, 0:D], v[h].rearrange("(t p) d -> p t d", p=P))
                vext_sb.append(vt)

            tri_sb = persist.tile([P, P], MMDT, tag="tri", name="tri_sb")
            nc.sync.dma_start(tri_sb[:], tri[:])


            # ---- main loops ----
            for s in range(NSTRIP):
                q0 = s * QW
                nt = 4 * s + 4  # kv tiles; last four straddle the diagonal
                # per-strip zn tiles: finer scheduler deps for the projection
                # stage + double-buffering across strips
                zn_sb = [znp.tile([P, QW], MMDT, tag=f"zn{j}", name=f"zn{j}") for j in range(2)]
                for h in range(H):
                    j, off = h // 2, (h % 2) * D
                    z_ps = zps.tile([D + 1, QW], F32, tag="z", name="z_ps")
                    # full kv tiles in pairs: two matmuls into a 2-bank PSUM
                    # tile, ONE exp over both (amortizes ACT per-op latency)
                    for t in range(0, 4 * s, 2):
                        sc2 = scps.tile([P, 2, QW], F32, tag="sc2", name="sc2")
                        for o in (0, 1):
                            nc.tensor.matmul(
                                sc2[:, o, :],
                                (kT_sb[j][off : off + D, (t + o) * P : (t + o + 1) * P]),
                                (qT_sb[j][off : off + D, q0 : q0 + QW]),
                                start=True,
                                stop=True,
                            )
                        ex2 = expp.tile([P, 2, QW], MMDT, tag="ex", name="ex")
                        nc.scalar.activation(
                            ex2[:], sc2[:],
                            mybir.ActivationFunctionType.Exp, scale=0.125,
                        )
                        for o in (0, 1):
                            nc.tensor.matmul(
                                z_ps[:],
                                (vext_sb[h][:, t + o, :]),
                                (ex2[:, o, :]),
                                start=(t + o == 0),
                                stop=False,
                            )
                    # the four diagonal kv tiles, partial ranges
                    for i in range(4):
                        t = 4 * s + i
                        li = i * P
                        sc2 = scps.tile([P, 2, QW], F32, tag="sc2", name="sc2")
                        nc.tensor.matmul(
                            sc2[:, 0, li:QW],
                            (kT_sb[j][off : off + D, t * P : (t + 1) * P]),
                            (qT_sb[j][off : off + D, q0 + li : q0 + QW]),
                            start=True,
                            stop=True,
                        )
                        ex2 = expp.tile([P, 2, QW], MMDT, tag="ex", name="ex")
                        nc.scalar.activation(
                            ex2[:, 0, li:QW], sc2[:, 0, li:QW],
                            mybir.ActivationFunctionType.Exp, scale=0.125,
                        )
                        nc.vector.tensor_mul(
                            ex2[:, 0, li : li + P], ex2[:, 0, li : li + P], tri_sb[:]
                        )
                        nc.tensor.matmul(
                            z_ps[:, li:QW],
                            (vext_sb[h][:, t, :]),
                            (ex2[:, 0, li:QW]),
                            start=(t == 0),
                            stop=(t == nt - 1),
                        )
                    # normalize: z[0:64] * (1 / z[64]); approx recip is ~5x
                    # faster than the 8-pass exact DVE reciprocal and exact to
                    # ~4e-6, far below the fp32r input rounding (2^-12)
                    dcp = rcpp.tile([1, QW], F32, tag="dcp", name="dcp")
                    nc.vector.tensor_copy(dcp[:], z_ps[D : D + 1, :])
                    rcp = rcpp.tile([1, QW], F32, tag="rcp", name="rcp")
                    # (custom-DVE op requires an SBUF input; PSUM reads garbage)
                    nc.vector.reciprocal_approx_fast(rcp[:], dcp[:])
                    # broadcast 1/denom across the 64 d-partitions via a DRAM
                    # bounce: DRAM sources allow a step-0 partition dim
                    rdr = dramp.tile([1, QW], F32, tag="rdr", name="rdr")
                    nc.sync.dma_start(rdr[:], rcp[:])
                    rb_sb = rcpp.tile([D, QW], F32, tag="rb_sb", name="rb_sb")
                    nc.sync.dma_start(
                        rb_sb[:],
                        bass.AP(tensor=rdr.tensor, offset=rdr.offset,
                                ap=[[0, D]] + [list(a) for a in rdr.ap][1:]),
                    )
                    nc.vector.tensor_mul(
                        zn_sb[j][off : off + D, :], z_ps[0:D, :], rb_sb[:]
                    )
                # output projection for this strip's four q blocks
                for qb in range(4 * s, 4 * s + 4):
                    for mt in range(2):
                        ops = wops.tile([P, 512], F32, tag="wo_ps", name="wo_ps")
                        for j2 in range(2):
                            nc.tensor.matmul(
                                ops[:],
                                (zn_sb[j2][:, (qb - 4 * s) * P : (qb - 4 * s + 1) * P]),
                                (wo_sb[j2][:, mt * 512 : (mt + 1) * 512]),
                                start=(j2 == 0),
                                stop=(j2 == 1),
                            )
                        ot = outp.tile([P, 512], F32, tag="ot", name="ot")
                        nc.vector.tensor_copy(ot[:], ops[:])
                        nc.sync.dma_start(
                            pout[qb * P : (qb + 1) * P, mt * 512 : (mt + 1) * 512], ot[:]
                        )

    nc.finalize()
    return nc


def _get_program():
    global _PROGRAM
    if _PROGRAM is None:
        _PROGRAM = build_program()
    return _PROGRAM


def make_in_maps(q, k, v, W_O, n_cores=N_CORES):
    """Shard full inputs into per-core maps (core = batch*4 + head_group)."""
    import ml_dtypes
    mmdt = ml_dtypes.bfloat16 if USE_BF16 else np.float32
    q = np.ascontiguousarray(np.asarray(q, dtype=np.float32))
    k = np.ascontiguousarray(np.asarray(k, dtype=np.float32))
    v = np.ascontiguousarray(np.asarray(v, dtype=np.float32))
    W_O = np.ascontiguousarray(np.asarray(W_O, dtype=np.float32))
    B = q.shape[0]
    qT = np.ascontiguousarray(q.reshape(B, S, N_HEADS, D).transpose(0, 2, 3, 1))
    kT = np.ascontiguousarray(k.reshape(B, S, N_HEADS, D).transpose(0, 2, 3, 1))
    vh = np.ascontiguousarray(v.reshape(B, S, N_HEADS, D).transpose(0, 2, 1, 3))
    # mask[kv, q] = 1 iff kv <= q  (scores live transposed: partition=kv, free=q)
    tri = np.ascontiguousarray(np.triu(np.ones((P, P), dtype=np.float32)))
    in_maps = []
    for core in range(n_cores):
        b, g = core // 4, core % 4
        hs = slice(H * g, H * (g + 1))
        in_maps.append(
            {
                "qT": np.ascontiguousarray(qT[b, hs]).astype(mmdt),
                "kT": np.ascontiguousarray(kT[b, hs]).astype(mmdt),
                "v": np.ascontiguousarray(vh[b, hs]).astype(mmdt),
                "wo": np.ascontiguousarray(W_O[hs].reshape(2 * P, D_MODEL)).astype(mmdt),
                "tri": tri.astype(mmdt),
            }
        )
    return in_maps


def kernel(residual, q, k, v, W_O, b_O, _trace=False, _trace_kwargs=None):
    global LAST_RESULTS
    residual = np.asarray(residual, dtype=np.float32)
    B = residual.shape[0]
    in_maps = make_in_maps(q, k, v, W_O)
    nc = _get_program()
    res = run_bass_kernel_spmd(
        nc, in_maps, list(range(N_CORES)), trace=_trace, **(_trace_kwargs or {})
    )
    LAST_RESULTS = res
    out = np.zeros((B, S, D_MODEL), dtype=np.float64)
    for core in range(N_CORES):
        out[core // 4] += res.results[core]["pout"].astype(np.float64)
    out += np.asarray(b_O, dtype=np.float64)
    return (residual, out.astype(np.float32))

